# revision 10
# baseline (speedup 1.0000x reference)
"""Trainium2 Bass kernel for nn_Adapter_CrossNonParam (adapter + prompt/token cross-attention).

Data-parallel over batch: 8 NeuronCores x 4 batches each, adapter weights
replicated, all matmuls bf16 (fp32 PSUM). x is pre-transposed/cast on the host.

v3: explicit per-batch software pipeline so DMA loads (batch b+1) and stores
(batch b) overlap for the whole run -- the DMA roofline for this kernel is
~37MB @ ~350GB/s ~= 105us/core, so every engine must stay below that span.

Emission order per iteration b:
    load x(b+2)                  [sync SWDGE ring]
    down(b+1): 5x8 matmuls into a single recycled PSUM bank, gelu per
               512-chunk (ACT, gelu table)
    attn(b):   per token-tile PAIR j (8): 2 logits MMs into one PSUM bank ->
               one [128,2,200] exp (ACT) -> PE transposes into a dedicated
               bf16 bank -> one pair toktr copy (DVE) -> per tile: 2 up MMs
               + one [128,1024] PSUM->SBUF cast (DVE/ACT mix) -> po pair MMs;
               exp partial sums: accA chain on DVE, accB chain on GpSimd
    tail(b):   partition-major denominator via 4 tiny matmuls, reciprocal,
               prompt up-proj, normalization folded into the epilogue scale
               (2 ACT muls + 2 DVE muls); runs under down(b+2).

ACT table discipline: per-instruction scheduler deps pin the ACT order to
G0 G1 E0 G2 E1 G3 E2 E3 (gelu/exp alternate once per batch -> ~8 table
loads instead of the 25-39 a free-running schedule produced; each load is
~1.3us of ACT time). ACT casts/muls are Copy-class = in every table set.

PSUM ledger (8 banks): down 1, logits(pair) 1, transpose(pair) 1,
up 4 (2 tiles x 2 banks), po+den 1.
"""
import numpy as np
import ml_dtypes

import concourse.bass as bass
import concourse.tile as tile
from concourse import bacc, mybir
from concourse.bass_utils import run_bass_kernel_spmd
from concourse.tile_rust import add_dep_helper

BF = mybir.dt.bfloat16
F32 = mybir.dt.float32

B, N, C = 32, 2248, 1024
D = 128
P = 200
T = N - P  # 2048
NCORES = 8
NB = B // NCORES  # 4 batches per core
SCALE = float(D) ** -0.5

CTILES = C // 128  # 8
TTILES = T // 128  # 16
DOWN_CHUNKS = [(s, min(512, N - s)) for s in range(0, N, 512)]  # 4x512 + 200

# epilogue cast engine per token tile: 5 ACT / 11 DVE (ACT also carries
# gelu+exp+tables; DVE the toktr copies and accA adds)
CAST_ON_ACT = {1, 4, 7, 10, 13}


def build_nc():
    nc = bacc.Bacc("TRN2", target_bir_lowering=False, debug=False, num_devices=NCORES)

    xT = nc.dram_tensor("xT", [NB, C, N], BF, kind="ExternalInput")
    wdn = nc.dram_tensor("wdn", [128, CTILES, 128], BF, kind="ExternalInput")
    wup = nc.dram_tensor("wup", [D, C], BF, kind="ExternalInput")
    bdn = nc.dram_tensor("bdn", [D, 1], F32, kind="ExternalInput")
    ident = nc.dram_tensor("ident", [128, 128], BF, kind="ExternalInput")
    onesf = nc.dram_tensor("onesf", [128, 1], F32, kind="ExternalInput")
    out = nc.dram_tensor("out", [NB, N, C], BF, kind="ExternalOutput")

    with tile.TileContext(nc) as tc:
        with (
            tc.tile_pool(name="const", bufs=1) as const,
            tc.tile_pool(name="xp", bufs=2) as xp,
            tc.tile_pool(name="dg", bufs=3) as dg,
            tc.tile_pool(name="ex", bufs=2) as ex,
            tc.tile_pool(name="tt", bufs=2) as tt,
            tc.tile_pool(name="red", bufs=1) as red,
            tc.tile_pool(name="sm", bufs=1) as sm,
            tc.tile_pool(name="ob", bufs=5) as ob,
            tc.tile_pool(name="ps_dn", bufs=1, space="PSUM") as ps_dn,
            tc.tile_pool(name="ps_lg", bufs=2, space="PSUM") as ps_lg,
            tc.tile_pool(name="ps_up", bufs=2, space="PSUM") as ps_up,
            tc.tile_pool(name="ps_po", bufs=1, space="PSUM") as ps_po,
        ):
            # ---- constants on the scalar ring ----
            wdn_sb = const.tile([128, CTILES, 128], BF)
            nc.scalar.dma_start(wdn_sb[:], wdn[:])
            bdn_sb = const.tile([D, 1], F32)
            nc.scalar.dma_start(bdn_sb[:], bdn[:])
            id_sb = const.tile([128, 128], BF)
            nc.scalar.dma_start(id_sb[:], ident[:])
            wup_sb = const.tile([D, C], BF)
            nc.scalar.dma_start(wup_sb[:], wup[:])
            onesf_sb = const.tile([128, 1], F32)
            nc.scalar.dma_start(onesf_sb[:], onesf[:])

            xsb_tiles = {}

            def load_x(b, split=False):
                """split=True: n-piece loads so down(0) starts on the first
                512 columns while the rest streams in."""
                xsb = xp.tile([128, CTILES, N], BF, tag="xsb")
                xsb_tiles[b] = xsb
                if split:
                    for s, w in ((0, 512), (512, 512), (1024, 512), (1536, 712)):
                        src_ap = xT[b, :, s : s + w].rearrange(
                            "(a p) n -> p a n", p=128
                        )
                        nc.sync.dma_start(xsb[:, :, s : s + w], src_ap)
                else:
                    for h in range(2):
                        src = xT[b, h * 512 : (h + 1) * 512, :].rearrange(
                            "(a p) n -> p a n", p=128
                        )
                        nc.sync.dma_start(xsb[:, h * 4 : (h + 1) * 4, :], src)

            def down(b):
                """Down-projection + gelu. Single PSUM bank recycled per
                512-chunk; the scheduler fills the gelu-wait gaps with the
                concurrent attn(b-1) matmuls."""
                xsb = xsb_tiles[b]
                dng = dg.tile([128, N], BF, tag="dng")
                gelus = []
                for s, w in DOWN_CHUNKS:
                    acc_full = ps_dn.tile([128, 512], F32, tag="dn")
                    acc = acc_full[:, :w]
                    for c in range(CTILES):
                        nc.tensor.matmul(
                            acc[:],
                            wdn_sb[:, c, :],
                            xsb[:, c, s : s + w],
                            start=(c == 0),
                            stop=(c == CTILES - 1),
                        )
                    g = nc.scalar.activation(
                        dng[:, s : s + w],
                        acc[:],
                        mybir.ActivationFunctionType.Gelu,
                        bias=bdn_sb[:],
                        scale=1.0,
                    )
                    gelus.append(g)
                return dng, gelus

            def attn(b, dng):
                exps = ex.tile([128, TTILES, P], BF, tag="exps")
                toktr = tt.tile([128, TTILES, 128], BF, tag="toktr")
                poT = ps_po.tile([128, P + 8], F32, tag="po")
                accA = red.tile([128, P], F32, tag="accA")
                accB = red.tile([128, P], F32, tag="accB")
                exp_insts = []
                osb = None
                for j in range(TTILES // 2):
                    t0, t1 = 2 * j, 2 * j + 1
                    lg = ps_lg.tile([128, 2, P], F32, tag="lg")
                    for k, t in ((0, t0), (1, t1)):
                        tok = dng[:, P + t * 128 : P + (t + 1) * 128]
                        nc.tensor.matmul(
                            lg[:, k, :], tok, dng[:, 0:P], start=True, stop=True
                        )
                    e = nc.scalar.activation(
                        exps[:, t0 : t0 + 2, :],
                        lg[:],
                        mybir.ActivationFunctionType.Exp,
                        scale=SCALE,
                    )
                    exp_insts.append(e)
                    # token-tile transposes on the scalar HWDGE ring's xbar
                    # (SBUF->SBUF, bf16): frees the PE matmul slots and the
                    # DVE copies entirely; scalar ring is idle after the
                    # constants so there is a single xbar-mode transition.
                    for k, t in ((0, t0), (1, t1)):
                        tok = dng[:, P + t * 128 : P + (t + 1) * 128]
                        nc.scalar.dma_start_transpose(toktr[:, t, :], tok)
                    # exp partial sums: accA chain on DVE, accB on GpSimd
                    if j == 0:
                        nc.vector.tensor_add(accA[:], exps[:, 0, :], exps[:, 1, :])
                    elif j == 1:
                        nc.gpsimd.tensor_add(accB[:], exps[:, 2, :], exps[:, 3, :])
                    else:
                        nc.vector.tensor_add(accA[:], accA[:], exps[:, t0, :])
                        nc.gpsimd.tensor_add(accB[:], accB[:], exps[:, t1, :])
                    # up-proj + one [128,1024] PSUM->SBUF cast per tile
                    for k, t in ((0, t0), (1, t1)):
                        tok = dng[:, P + t * 128 : P + (t + 1) * 128]
                        q = t % 4
                        if q == 0:
                            osb = ob.tile([128, 4, 2, 512], BF, tag="osb")
                        up = ps_up.tile([128, 2, 512], F32, tag="up")
                        for h in range(2):
                            nc.tensor.matmul(
                                up[:, h, :],
                                tok,
                                wup_sb[:, h * 512 : (h + 1) * 512],
                                start=True,
                                stop=True,
                            )
                        up_flat = up[:].rearrange("p a b -> p (a b)")
                        dst = osb[:, q].rearrange("p a b -> p (a b)")
                        if t in CAST_ON_ACT:
                            nc.scalar.copy(dst, up_flat)
                        else:
                            nc.vector.tensor_copy(dst, up_flat)
                        if q == 3:
                            g4 = t // 4
                            dstd = out[
                                b, P + 512 * g4 : P + 512 * (g4 + 1), :
                            ].rearrange("(a p) c -> p a c", p=128)
                            nc.gpsimd.dma_start(
                                dstd, osb[:].rearrange("p a b c -> p a (b c)")
                            )
                # po accumulation as one contiguous block: by now every
                # toktr/exps producer is many pairs ahead, so the PE never
                # stalls on them (measured ~680ns per po MM when interleaved)
                for t in range(TTILES):
                    nc.tensor.matmul(
                        poT[:, 0:P],
                        toktr[:, t, :],
                        exps[:, t, :],
                        start=(t == 0),
                        stop=(t == TTILES - 1),
                    )
                return poT, accA, accB, exp_insts

            def tail(b, dng, poT, accA, accB):
                """Attention tail: denominator, reciprocal, prompt up-proj
                with normalization folded into the epilogue scale."""
                # accA += accB on GpSimd (SBUF-only) halves the den matmuls
                nc.gpsimd.tensor_add(accA[:], accA[:], accB[:])
                nc.tensor.matmul(
                    poT[:, P : P + 1], accA[:, 0:128], onesf_sb[:],
                    start=True, stop=True,
                )
                nc.tensor.matmul(
                    poT[0:72, P + 1 : P + 2], accA[:, 128:200], onesf_sb[:],
                    start=True, stop=True,
                )
                rec0 = sm.tile([128, 1], F32, tag="rec0")
                nc.vector.reciprocal(rec0[:], poT[:, P : P + 1])
                rec1 = sm.tile([72, 1], F32, tag="rec1")
                nc.vector.reciprocal(rec1[:], poT[0:72, P + 1 : P + 2])
                # unnormalized prompt_out -> dng's prompt region (DVE)
                nc.vector.tensor_copy(dng[:, 0:P], poT[:, 0:P])

                osbp = ob.tile([128, 4, 2, 512], BF, tag="osb")
                for h in range(2):
                    up = ps_up.tile([128, 2, 512], F32, tag="up")
                    nc.tensor.matmul(
                        up[:, 0, :],
                        dng[:, 0:128],
                        wup_sb[:, h * 512 : (h + 1) * 512],
                        start=True, stop=True,
                    )
                    nc.scalar.mul(osbp[:, 0, h, :], up[:, 0, :], rec0[:])
                    nc.tensor.matmul(
                        up[0:72, 1, :],
                        dng[:, 128:200],
                        wup_sb[:, h * 512 : (h + 1) * 512],
                        start=True, stop=True,
                    )
                    nc.vector.tensor_scalar_mul(
                        osbp[0:72, 1, h, :], up[0:72, 1, :], rec1[:]
                    )
                nc.gpsimd.dma_start(
                    out[b, 0:128, :], osbp[:, 0].rearrange("p a b -> p (a b)")
                )
                nc.gpsimd.dma_start(
                    out[b, 128:200, :], osbp[0:72, 1].rearrange("p a b -> p (a b)")
                )

            # ---- software pipeline ----
            load_x(0, split=True)
            load_x(1)
            dngs = {}
            gelu_groups = {}
            exp_groups = {}
            dngs[0], gelu_groups[0] = down(0)
            for b in range(NB):
                if b + 2 < NB:
                    load_x(b + 2)
                if b + 1 < NB:
                    dngs[b + 1], gelu_groups[b + 1] = down(b + 1)
                poT, accA, accB, exp_groups[b] = attn(b, dngs[b])
                tail(b, dngs[b], poT, accA, accB)

            # ACT table discipline: pin the per-engine order
            # G0 G1 E0 G2 E1 G3 E2 E3 so Gelu/Exp alternate once per batch.
            # Per-instruction edges -- pinning only the group heads lets the
            # scheduler interleave the remaining instructions (measured: 39
            # table loads instead of 8).
            for b in range(NB):
                if b + 1 < NB:
                    g_last = gelu_groups[b + 1][-1].ins
                    for e in exp_groups[b]:
                        add_dep_helper(
                            e.ins, g_last, sync=False,
                            reason="ACT order: exps(b) after gelus(b+1)",
                        )
                if b + 2 < NB:
                    e_last = exp_groups[b][-1].ins
                    for g in gelu_groups[b + 2]:
                        add_dep_helper(
                            g.ins, e_last, sync=False,
                            reason="ACT order: gelus(b+2) after exps(b)",
                        )

    nc.compile()
    return nc


_NC_CACHE = None


def _get_nc():
    global _NC_CACHE
    if _NC_CACHE is None:
        _NC_CACHE = build_nc()
    return _NC_CACHE


def make_in_maps(x, W_down, b_down, W_up, b_up, gate):
    x = np.asarray(x, np.float32)
    W_down = np.asarray(W_down, np.float32)
    b_down = np.asarray(b_down, np.float32)
    W_up = np.asarray(W_up, np.float32)
    b_up = np.asarray(b_up, np.float32)
    gate = float(np.asarray(gate, np.float32))

    bf = ml_dtypes.bfloat16
    xT = np.ascontiguousarray(x.transpose(0, 2, 1)).astype(bf)  # [B, C, N]
    # wdn[p, c, m] = W_down[c*128 + p, m]
    wdn = np.ascontiguousarray(
        W_down.reshape(CTILES, 128, 128).transpose(1, 0, 2)
    ).astype(bf)
    wup = (W_up * gate).astype(bf)  # [D, C]
    bdn = b_down.reshape(D, 1).copy()
    ident = np.eye(128, dtype=bf)
    onesf = np.ones((128, 1), dtype=np.float32)

    in_maps = []
    for i in range(NCORES):
        in_maps.append(
            {
                "xT": np.ascontiguousarray(xT[i * NB : (i + 1) * NB]),
                "wdn": wdn,
                "wup": wup,
                "bdn": bdn,
                "ident": ident,
                "onesf": onesf,
            }
        )
    return in_maps


def kernel(**inputs):
    nc = _get_nc()
    in_maps = make_in_maps(**inputs)
    res = run_bass_kernel_spmd(nc, in_maps, core_ids=list(range(NCORES)))
    out = np.concatenate([res.results[i]["out"] for i in range(NCORES)], axis=0)
    out = out.astype(np.float32)
    # b_up (and gate) folded in on the host: device computes comb @ (gate*W_up)
    bias = (
        np.asarray(inputs["b_up"], np.float32)
        * float(np.asarray(inputs["gate"], np.float32))
    ).reshape(1, 1, C)
    return out + bias


# revision 13
# speedup vs baseline: 1.7905x; 1.7905x over previous
"""Trainium2 Bass kernel for nn_Adapter_CrossNonParam (adapter + prompt/token cross-attention).

Data-parallel over batch: 8 NeuronCores x 4 batches each, adapter weights
replicated, all matmuls bf16 (fp32 PSUM). x is pre-transposed/cast on the host.

v3: explicit per-batch software pipeline so DMA loads (batch b+1) and stores
(batch b) overlap for the whole run -- the DMA roofline for this kernel is
~37MB @ ~350GB/s ~= 105us/core, so every engine must stay below that span.

Emission order per iteration b:
    load x(b+2)                  [sync SWDGE ring]
    down(b+1): 5x8 matmuls into a single recycled PSUM bank, gelu per
               512-chunk (ACT, gelu table)
    attn(b):   per token-tile PAIR j (8): 2 logits MMs into one PSUM bank ->
               one [128,2,200] exp (ACT) -> PE transposes into a dedicated
               bf16 bank -> one pair toktr copy (DVE) -> per tile: 2 up MMs
               + one [128,1024] PSUM->SBUF cast (DVE/ACT mix) -> po pair MMs;
               exp partial sums: accA chain on DVE, accB chain on GpSimd
    tail(b):   partition-major denominator via 4 tiny matmuls, reciprocal,
               prompt up-proj, normalization folded into the epilogue scale
               (2 ACT muls + 2 DVE muls); runs under down(b+2).

ACT table discipline: per-instruction scheduler deps pin the ACT order to
G0 G1 E0 G2 E1 G3 E2 E3 (gelu/exp alternate once per batch -> ~8 table
loads instead of the 25-39 a free-running schedule produced; each load is
~1.3us of ACT time). ACT casts/muls are Copy-class = in every table set.

PSUM ledger (8 banks): down 1, logits(pair) 1, transpose(pair) 1,
up 4 (2 tiles x 2 banks), po+den 1.
"""
import numpy as np
import ml_dtypes

import concourse.bass as bass
import concourse.tile as tile
from concourse import bacc, mybir
from concourse.bass_utils import run_bass_kernel_spmd
from concourse.tile_rust import add_dep_helper

BF = mybir.dt.bfloat16
F32 = mybir.dt.float32

B, N, C = 32, 2248, 1024
D = 128
P = 200
T = N - P  # 2048
NCORES = 8
NB = B // NCORES  # 4 batches per core
SCALE = float(D) ** -0.5

CTILES = C // 128  # 8
TTILES = T // 128  # 16
DOWN_CHUNKS = [(s, min(512, N - s)) for s in range(0, N, 512)]  # 4x512 + 200

# epilogue cast engine per token tile: 5 ACT / 11 DVE (ACT also carries
# gelu+exp+tables; DVE the toktr copies and accA adds)
CAST_ON_ACT = {1, 4, 7, 10, 13}


def build_nc():
    nc = bacc.Bacc("TRN2", target_bir_lowering=False, debug=False, num_devices=NCORES)

    xT = nc.dram_tensor("xT", [NB, C, N], BF, kind="ExternalInput")
    wdn = nc.dram_tensor("wdn", [128, CTILES, 128], BF, kind="ExternalInput")
    wup = nc.dram_tensor("wup", [D, C], BF, kind="ExternalInput")
    bdn = nc.dram_tensor("bdn", [D, 1], F32, kind="ExternalInput")
    ident = nc.dram_tensor("ident", [128, 128], BF, kind="ExternalInput")
    onesf = nc.dram_tensor("onesf", [128, 1], F32, kind="ExternalInput")
    out = nc.dram_tensor("out", [NB, N, C], BF, kind="ExternalOutput")

    with tile.TileContext(nc) as tc:
        with (
            tc.tile_pool(name="const", bufs=1) as const,
            tc.tile_pool(name="xp", bufs=2) as xp,
            tc.tile_pool(name="dg", bufs=3) as dg,
            tc.tile_pool(name="ex", bufs=2) as ex,
            tc.tile_pool(name="tt", bufs=2) as tt,
            tc.tile_pool(name="red", bufs=1) as red,
            tc.tile_pool(name="sm", bufs=1) as sm,
            tc.tile_pool(name="ob", bufs=5) as ob,
            tc.tile_pool(name="ps_dn", bufs=1, space="PSUM") as ps_dn,
            tc.tile_pool(name="ps_lg", bufs=1, space="PSUM") as ps_lg,
            tc.tile_pool(name="ps_tr", bufs=1, space="PSUM") as ps_tr,
            tc.tile_pool(name="ps_up", bufs=2, space="PSUM") as ps_up,
            tc.tile_pool(name="ps_po", bufs=1, space="PSUM") as ps_po,
        ):
            # ---- constants on the scalar ring ----
            wdn_sb = const.tile([128, CTILES, 128], BF)
            nc.scalar.dma_start(wdn_sb[:], wdn[:])
            bdn_sb = const.tile([D, 1], F32)
            nc.scalar.dma_start(bdn_sb[:], bdn[:])
            id_sb = const.tile([128, 128], BF)
            nc.scalar.dma_start(id_sb[:], ident[:])
            wup_sb = const.tile([D, C], BF)
            nc.scalar.dma_start(wup_sb[:], wup[:])
            onesf_sb = const.tile([128, 1], F32)
            nc.scalar.dma_start(onesf_sb[:], onesf[:])

            xsb_tiles = {}

            def load_x(b, split=False):
                """split=True: n-piece loads so down(0) starts on the first
                512 columns while the rest streams in."""
                xsb = xp.tile([128, CTILES, N], BF, tag="xsb")
                xsb_tiles[b] = xsb
                if split:
                    for s, w in ((0, 512), (512, 512), (1024, 512), (1536, 712)):
                        src_ap = xT[b, :, s : s + w].rearrange(
                            "(a p) n -> p a n", p=128
                        )
                        nc.sync.dma_start(xsb[:, :, s : s + w], src_ap)
                else:
                    for h in range(2):
                        src = xT[b, h * 512 : (h + 1) * 512, :].rearrange(
                            "(a p) n -> p a n", p=128
                        )
                        nc.sync.dma_start(xsb[:, h * 4 : (h + 1) * 4, :], src)

            def down(b):
                """Down-projection + gelu. Single PSUM bank recycled per
                512-chunk; the scheduler fills the gelu-wait gaps with the
                concurrent attn(b-1) matmuls."""
                xsb = xsb_tiles[b]
                dng = dg.tile([128, N], BF, tag="dng")
                gelus = []
                for s, w in DOWN_CHUNKS:
                    acc_full = ps_dn.tile([128, 512], F32, tag="dn")
                    acc = acc_full[:, :w]
                    for c in range(CTILES):
                        nc.tensor.matmul(
                            acc[:],
                            wdn_sb[:, c, :],
                            xsb[:, c, s : s + w],
                            start=(c == 0),
                            stop=(c == CTILES - 1),
                        )
                    g = nc.scalar.activation(
                        dng[:, s : s + w],
                        acc[:],
                        mybir.ActivationFunctionType.Gelu,
                        bias=bdn_sb[:],
                        scale=1.0,
                    )
                    gelus.append(g)
                return dng, gelus

            def attn(b, dng):
                exps = ex.tile([128, TTILES, P], BF, tag="exps")
                toktr = tt.tile([128, TTILES, 128], BF, tag="toktr")
                poT = ps_po.tile([128, P + 8], F32, tag="po")
                accA = red.tile([128, P], F32, tag="accA")
                accB = red.tile([128, P], F32, tag="accB")
                exp_insts = []
                osb = None
                for j in range(TTILES // 2):
                    t0, t1 = 2 * j, 2 * j + 1
                    lg = ps_lg.tile([128, 2, P], F32, tag="lg")
                    for k, t in ((0, t0), (1, t1)):
                        tok = dng[:, P + t * 128 : P + (t + 1) * 128]
                        nc.tensor.matmul(
                            lg[:, k, :], tok, dng[:, 0:P], start=True, stop=True
                        )
                    e = nc.scalar.activation(
                        exps[:, t0 : t0 + 2, :],
                        lg[:],
                        mybir.ActivationFunctionType.Exp,
                        scale=SCALE,
                    )
                    exp_insts.append(e)
                    # pair of PE transposes into the dedicated bf16 bank,
                    # one DVE copy for both (DMA-xbar transpose measured
                    # 1.2us of issuing-engine time each -- not viable)
                    trp = ps_tr.tile([128, 2, 128], BF, tag="tr")
                    for k, t in ((0, t0), (1, t1)):
                        tok = dng[:, P + t * 128 : P + (t + 1) * 128]
                        nc.tensor.transpose(trp[:, k, :], tok, id_sb[:])
                    nc.vector.tensor_copy(toktr[:, t0 : t0 + 2, :], trp[:])
                    # exp partial sums: accA chain on DVE, accB on GpSimd
                    if j == 0:
                        nc.vector.tensor_add(accA[:], exps[:, 0, :], exps[:, 1, :])
                    elif j == 1:
                        nc.gpsimd.tensor_add(accB[:], exps[:, 2, :], exps[:, 3, :])
                    else:
                        nc.vector.tensor_add(accA[:], accA[:], exps[:, t0, :])
                        nc.gpsimd.tensor_add(accB[:], accB[:], exps[:, t1, :])
                    # up-proj + one [128,1024] PSUM->SBUF cast per tile
                    for k, t in ((0, t0), (1, t1)):
                        tok = dng[:, P + t * 128 : P + (t + 1) * 128]
                        q = t % 4
                        if q == 0:
                            osb = ob.tile([128, 4, 2, 512], BF, tag="osb")
                        up = ps_up.tile([128, 2, 512], F32, tag="up")
                        for h in range(2):
                            nc.tensor.matmul(
                                up[:, h, :],
                                tok,
                                wup_sb[:, h * 512 : (h + 1) * 512],
                                start=True,
                                stop=True,
                            )
                        # split the PSUM->SBUF cast across ACT and DVE in
                        # parallel: halves the bank-release latency so the
                        # next tile's up MMs stall less on the ps_up pool
                        ha = t % 2
                        nc.scalar.copy(osb[:, q, ha, :], up[:, ha, :])
                        nc.vector.tensor_copy(osb[:, q, 1 - ha, :], up[:, 1 - ha, :])
                        if q == 3:
                            g4 = t // 4
                            dstd = out[
                                b, P + 512 * g4 : P + 512 * (g4 + 1), :
                            ].rearrange("(a p) c -> p a c", p=128)
                            nc.gpsimd.dma_start(
                                dstd, osb[:].rearrange("p a b c -> p a (b c)")
                            )
                # po accumulation as one contiguous block: by now every
                # toktr/exps producer is many pairs ahead, so the PE never
                # stalls on them (measured ~680ns per po MM when interleaved)
                for t in range(TTILES):
                    nc.tensor.matmul(
                        poT[:, 0:P],
                        toktr[:, t, :],
                        exps[:, t, :],
                        start=(t == 0),
                        stop=(t == TTILES - 1),
                    )
                return poT, accA, accB, exp_insts

            def tail(b, dng, poT, accA, accB):
                """Attention tail: denominator, reciprocal, prompt up-proj
                with normalization folded into the epilogue scale."""
                # accA += accB on GpSimd (SBUF-only) halves the den matmuls
                nc.gpsimd.tensor_add(accA[:], accA[:], accB[:])
                nc.tensor.matmul(
                    poT[:, P : P + 1], accA[:, 0:128], onesf_sb[:],
                    start=True, stop=True,
                )
                nc.tensor.matmul(
                    poT[0:72, P + 1 : P + 2], accA[:, 128:200], onesf_sb[:],
                    start=True, stop=True,
                )
                rec0 = sm.tile([128, 1], F32, tag="rec0")
                nc.vector.reciprocal(rec0[:], poT[:, P : P + 1])
                rec1 = sm.tile([72, 1], F32, tag="rec1")
                nc.vector.reciprocal(rec1[:], poT[0:72, P + 1 : P + 2])
                # unnormalized prompt_out -> dng's prompt region (DVE)
                nc.vector.tensor_copy(dng[:, 0:P], poT[:, 0:P])

                osbp = ob.tile([128, 4, 2, 512], BF, tag="osb")
                for h in range(2):
                    up = ps_up.tile([128, 2, 512], F32, tag="up")
                    nc.tensor.matmul(
                        up[:, 0, :],
                        dng[:, 0:128],
                        wup_sb[:, h * 512 : (h + 1) * 512],
                        start=True, stop=True,
                    )
                    nc.scalar.mul(osbp[:, 0, h, :], up[:, 0, :], rec0[:])
                    nc.tensor.matmul(
                        up[0:72, 1, :],
                        dng[:, 128:200],
                        wup_sb[:, h * 512 : (h + 1) * 512],
                        start=True, stop=True,
                    )
                    nc.vector.tensor_scalar_mul(
                        osbp[0:72, 1, h, :], up[0:72, 1, :], rec1[:]
                    )
                nc.gpsimd.dma_start(
                    out[b, 0:128, :], osbp[:, 0].rearrange("p a b -> p (a b)")
                )
                nc.gpsimd.dma_start(
                    out[b, 128:200, :], osbp[0:72, 1].rearrange("p a b -> p (a b)")
                )

            # ---- software pipeline ----
            load_x(0, split=True)
            load_x(1)
            dngs = {}
            gelu_groups = {}
            exp_groups = {}
            dngs[0], gelu_groups[0] = down(0)
            for b in range(NB):
                if b + 2 < NB:
                    load_x(b + 2)
                if b + 1 < NB:
                    dngs[b + 1], gelu_groups[b + 1] = down(b + 1)
                poT, accA, accB, exp_groups[b] = attn(b, dngs[b])
                tail(b, dngs[b], poT, accA, accB)

            # ACT table discipline: pin the per-engine order
            # G0 G1 E0 G2 E1 G3 E2 E3 so Gelu/Exp alternate once per batch.
            # Per-instruction edges -- pinning only the group heads lets the
            # scheduler interleave the remaining instructions (measured: 39
            # table loads instead of 8).
            for b in range(NB):
                if b + 1 < NB:
                    g_last = gelu_groups[b + 1][-1].ins
                    for e in exp_groups[b]:
                        add_dep_helper(
                            e.ins, g_last, sync=False,
                            reason="ACT order: exps(b) after gelus(b+1)",
                        )
                if b + 2 < NB:
                    e_last = exp_groups[b][-1].ins
                    for g in gelu_groups[b + 2]:
                        add_dep_helper(
                            g.ins, e_last, sync=False,
                            reason="ACT order: gelus(b+2) after exps(b)",
                        )

    nc.compile()
    return nc


_NC_CACHE = None


def _get_nc():
    global _NC_CACHE
    if _NC_CACHE is None:
        _NC_CACHE = build_nc()
    return _NC_CACHE


def make_in_maps(x, W_down, b_down, W_up, b_up, gate):
    x = np.asarray(x, np.float32)
    W_down = np.asarray(W_down, np.float32)
    b_down = np.asarray(b_down, np.float32)
    W_up = np.asarray(W_up, np.float32)
    b_up = np.asarray(b_up, np.float32)
    gate = float(np.asarray(gate, np.float32))

    bf = ml_dtypes.bfloat16
    xT = np.ascontiguousarray(x.transpose(0, 2, 1)).astype(bf)  # [B, C, N]
    # wdn[p, c, m] = W_down[c*128 + p, m]
    wdn = np.ascontiguousarray(
        W_down.reshape(CTILES, 128, 128).transpose(1, 0, 2)
    ).astype(bf)
    wup = (W_up * gate).astype(bf)  # [D, C]
    bdn = b_down.reshape(D, 1).copy()
    ident = np.eye(128, dtype=bf)
    onesf = np.ones((128, 1), dtype=np.float32)

    in_maps = []
    for i in range(NCORES):
        in_maps.append(
            {
                "xT": np.ascontiguousarray(xT[i * NB : (i + 1) * NB]),
                "wdn": wdn,
                "wup": wup,
                "bdn": bdn,
                "ident": ident,
                "onesf": onesf,
            }
        )
    return in_maps


def kernel(**inputs):
    nc = _get_nc()
    in_maps = make_in_maps(**inputs)
    res = run_bass_kernel_spmd(nc, in_maps, core_ids=list(range(NCORES)))
    out = np.concatenate([res.results[i]["out"] for i in range(NCORES)], axis=0)
    out = out.astype(np.float32)
    # b_up (and gate) folded in on the host: device computes comb @ (gate*W_up)
    bias = (
        np.asarray(inputs["b_up"], np.float32)
        * float(np.asarray(inputs["gate"], np.float32))
    ).reshape(1, 1, C)
    return out + bias
